# revision 11
# baseline (speedup 1.0000x reference)
"""Chamfer loss kernel for Trainium2 (8 NeuronCores, data-parallel over batch).

Problem: B=8, N=M=4096, D=3 fp32 point clouds.
  loss = mean_b mean_n min_m ||p_bn - g_bm||^2  +  mean_b mean_m min_n ||.||^2
  (squared euclidean, clamped at 0, matching pytorch3d norm=2 semantics)

Strategy (one batch element per core):
  - Distance tiles d[n, m] = ||p||^2 + ||g||^2 - 2 p.g come from K=7 float32r
    matmuls on the PE. float32r keeps ~12 mantissa bits, so the points are
    rounded once (consistently) and the squared norms are computed FROM the
    rounded points, split into hi+lo f32r rows so the norm contribution keeps
    full fp32 precision. Row pairing (lhsT row k x rhs row k):
       k0: 1 * |g|^2_hi   k1: 1 * |g|^2_lo
       k2: |p|^2_hi * 1   k3: |p|^2_lo * 1
       k4-6: (-2 p_d) * g_d
    The resulting d error is only the point-rounding perturbation (~2e-4
    relative on coordinates), which moves the final loss by ~1e-5 relative.
  - The K=7 strips are replicated at PE row-group partition bases
    {0, 32, 64, 96} and 4 matmuls run CONCURRENTLY via tile_position row
    tiling (measured ~59us for all 256 [128, 512] tiles vs 456us for fp32).
  - Row minima: fused vector.tensor_scalar reads each 4-bank PSUM span once:
    out = relu(d) cast to bf16 into SBUF (feeds the column path), accum_out =
    running min along free, chained across the two spans per n-tile.
  - Column minima: bf16 tensor_tensor min into colacc[128, 4096], then a
    PE-transpose + free-axis reduce tail for the partition-axis min.
  - Per-core scalar output (cham_x_b + cham_y_b); the host averages the 8
    per-core scalars (the data-parallel gather).

All arithmetic happens on-chip; the host only reshapes/transposes inputs
(layout) and averages the per-core partial losses (unshard).
"""

import os
import sys

import numpy as np

sys.path.insert(0, "/opt/trn_rl_repo")

import bass_rust
import concourse.bass as bass
import concourse.mybir as mybir
from concourse.bass_utils import run_bass_kernel_spmd
from concourse.masks import make_identity
from concourse.tile import TileContext

B, N, M, D = 8, 4096, 4096, 3
NT = N // 128  # 32 n-tiles
K = 7
F32 = mybir.dt.float32
F32R = mybir.dt.float32r
BF16 = mybir.dt.bfloat16
BIG = 3.0e38

# ---------------------------------------------------------------------------
# walrus in this container rejects >1 sync-wait per instruction; spill the
# extras onto engine-matched NoOps placed immediately before the instruction.
_nop_counter = [0]


def _split_multi_waits(nc):
    for func in nc.m.functions:
        for bb in func.blocks:
            out = []
            dirty = False
            for inst in bb.instructions:
                si = inst.sync_info
                if si is not None and len(si.on_wait) > 1:
                    waits = list(si.on_wait)
                    for w in waits[:-1]:
                        _nop_counter[0] += 1
                        nop = mybir.InstNoOp(
                            name=f"I-waitsplit-{_nop_counter[0]}", ins=[], outs=[]
                        )
                        nop.engine = inst.engine
                        nop.sync_info = bass_rust.SyncInfo(on_wait=[w], on_update=[])
                        out.append(nop)
                    inst.sync_info = bass_rust.SyncInfo(
                        on_wait=[waits[-1]], on_update=list(si.on_update)
                    )
                    dirty = True
                out.append(inst)
            if dirty:
                bb.instructions = out
    return nc


# ---------------------------------------------------------------------------


_PREP_WR = {}


def _build_prep_side(nc, tc, pool, zpk, w_dram, ident, scale, hi_row, lo_row, c_row, q, psp):
    """Build one side's K=7 rows in row group 0 of zpk [96+K, 4096] f32r.

    Everything derives from the wide input layout [128, 96] (point 128t+p at
    partition p, cols 3t..3t+2): rounded coords via PE-transpose, squared
    norms (of the rounded points) hi/lo-split, both flattened into n-order
    by ONE reshape DMA each on this side's HWDGE queue (requires lo_row ==
    hi_row + 1). Row groups 1-3 are filled by the replication hops in
    build_nc; keeping the DMA count tiny is what keeps the prep ramp short.
    """
    nm = w_dram.name
    wr = _PREP_WR[nm]  # rounded+scaled wide input, loaded up front

    if True:
        # --- coords: transpose wr -> [96, 128], one 3-row reshape DMA ---
        tw_ps = psp.tile([128, 128], F32, name=f"twps_{nm}", tag="ps_main")
        nc.tensor.matmul(
            tw_ps[0 : 3 * NT, :],
            wr.bitcast(F32),
            ident,
            is_transpose=True,
            start=True,
            stop=True,
        )
        tw = pool.tile([3 * NT, 128], F32R, name=f"tw_{nm}")
        nc.vector.tensor_copy(tw, tw_ps[0 : 3 * NT, :])
        tw_d = tw.rearrange("(t d) p -> d t p", d=3)
        for d in range(3):
            q.dma_start(out=zpk[c_row + d : c_row + d + 1, :], in_=tw_d[d])

        # --- norms of rounded points ---
        wsq = pool.tile([128, 3 * NT], F32, name=f"wsq_{nm}")
        nc.vector.tensor_mul(wsq, wr, wr)
        norms = pool.tile([128, NT], F32, name=f"norms_{nm}")
        nc.vector.tensor_reduce(
            out=norms,
            in_=wsq.rearrange("p (t d) -> p t d", d=3),
            axis=mybir.AxisListType.X,
            op=mybir.AluOpType.add,
        )
        if scale != 1.0:
            # norms of scale*p -> divide by scale^2 (exact for powers of 2)
            nc.vector.tensor_scalar(
                out=norms,
                in0=norms,
                scalar1=1.0 / (scale * scale),
                scalar2=None,
                op0=mybir.AluOpType.mult,
            )
        nh = pool.tile([128, NT], F32R, name=f"nh_{nm}")
        nc.vector.tensor_copy(nh, norms)
        nl_f = pool.tile([128, NT], F32, name=f"nlf_{nm}")
        nc.vector.tensor_sub(nl_f, norms, nh.bitcast(F32))
        nl = pool.tile([128, NT], F32R, name=f"nl_{nm}")
        nc.vector.tensor_copy(nl, nl_f)
        assert lo_row == hi_row + 1
        tn2 = pool.tile([2 * NT, 128], F32R, name=f"tn2_{nm}")
        for i, src in enumerate((nh, nl)):
            tn_ps = psp.tile([128, 128], F32, name=f"tnps_{nm}_{i}", tag="ps_main")
            nc.tensor.matmul(
                tn_ps[0:NT, :],
                src.bitcast(F32),
                ident,
                is_transpose=True,
                start=True,
                stop=True,
            )
            nc.vector.tensor_copy(tn2[NT * i : NT * (i + 1), :], tn_ps[0:NT, :])
        q.dma_start(out=zpk[hi_row : hi_row + 2, :], in_=tn2)


def build_nc():
    nc = bass.Bass("TRN2")
    predW = nc.dram_tensor("predW", [128, 3 * NT], F32, kind="ExternalInput")
    gtW = nc.dram_tensor("gtW", [128, 3 * NT], F32, kind="ExternalInput")
    out_d = nc.dram_tensor("out", [1, 1], F32, kind="ExternalOutput")

    with TileContext(nc) as tc:
        with (
            tc.tile_pool(name="persist", bufs=1) as persist,
            tc.tile_pool(name="dsb", bufs=2) as dsbp,
        ):
            # load + round both wide inputs first — everything derives from
            # them, so they must not queue behind prep DMAs
            for w_dram, scale, q in ((predW, -2.0, nc.sync), (gtW, 1.0, nc.scalar)):
                w_in = persist.tile([128, 3 * NT], F32, name=f"w_{w_dram.name}")
                q.dma_start(out=w_in, in_=w_dram.ap())
                wr_t = persist.tile([128, 3 * NT], F32R, name=f"wr_{w_dram.name}")
                nc.scalar.mul(out=wr_t, in_=w_in, mul=scale)
                _PREP_WR[w_dram.name] = wr_t
            # identity zero-fill on DVE so GPSIMD's single queue only does
            # the diagonal writes (keeps the prep critical path short)
            ident = persist.tile([128, 128], F32)
            nc.vector.memset(ident, 0.0)
            make_identity(nc, ident, nomemset=True)
            ident16 = persist.tile([128, 128], BF16)
            nc.vector.memset(ident16, 0.0)
            make_identity(nc, ident16, nomemset=True)

            # K=7 operand strips, replicated at the 4 PE row-group bases so
            # 4 matmuls stream concurrently (PE ~59us vs ~200us unpacked
            # in-situ; LDWEIGHTS overlaps across row groups).
            zp_pk = persist.tile([96 + K, N], F32R)
            zg_pk = persist.tile([96 + K, M], F32R)
            # ones rows (Memset can't target f32r; compute ops can't start at
            # unaligned partitions): rounded ones strip -> broadcast DMA.
            # ones rows, built wide (128 lanes, ~100ns) then DMA'd out
            ones_w = persist.tile([128, 64], F32)
            nc.vector.memset(ones_w, 1.0)
            ones_wr = persist.tile([128, 64], F32R)
            nc.scalar.copy(ones_wr, ones_w)
            nc.sync.dma_start(out=zp_pk[0:2, :], in_=ones_wr)
            nc.scalar.dma_start(out=zg_pk[2:4, :], in_=ones_wr)
            # zp rows: [1, 1, nPh, nPl, -2p0, -2p1, -2p2]
            # zg rows: [nGh, nGl, 1, 1, g0, g1, g2]
            psp = tc.alloc_tile_pool(name="psum_all", bufs=2, space="PSUM")
            _build_prep_side(
                nc, tc, persist, zp_pk, predW, ident, -2.0, 2, 3, 4,
                nc.sync, psp,
            )
            _build_prep_side(
                nc, tc, persist, zg_pk, gtW, ident, 1.0, 0, 1, 4,
                nc.scalar, psp,
            )
            # replicate group 0 -> groups 1-3 (one [K, 4096] block copy per
            # group, all sourcing g0 so the three hops pipeline per queue;
            # zp on sync, zg on scalar run in parallel)
            for zpk, q in ((zp_pk, nc.sync), (zg_pk, nc.scalar)):
                for g in (1, 2, 3):
                    q.dma_start(out=zpk[32 * g : 32 * g + K, :], in_=zpk[0:K, :])

            rowmins = persist.tile([128, NT], F32)
            # two independent column accumulators (updated as one [128,2,M]
            # tensor_tensor, halving the op count) merged once at the end
            colacc2 = persist.tile([128, 2 * M], BF16)
            colacc = persist.tile([128, M], BF16)
            colacc2_v = colacc2.rearrange("p (i m) -> p i m", m=M)
            # fold-tree intermediates: written+read only by the DVE (in-order)
            # so single buffers suffice; sized for the max group of 4 tiles
            foldp = tc.alloc_tile_pool(name="foldp", bufs=1)
            ftiles = [
                foldp.tile([128, 4, w], BF16, name=f"f{w}") for w in (2048, 1024, 512, 256, 128, 64)
            ]

            # ---- main loop: 32 n-tiles x 2 half-spans of [128, 2048];
            #      each span = 4 row-group-packed concurrent matmuls ----
            # Pipeline per n-tile: PE (4 packed MMs per half-span) -> ACT
            # relu-copies PSUM->SBUF bf16 (clamp fused, so no tail clamps)
            # -> DVE row path: deep bf16 fold-tree (2x mode) over groups of
            # 4 tiles down to width 64 before the (1x-rate) tensor_reduce;
            # col path: [128,2,M] tensor_tensor min into the paired
            # accumulators. Steady-state DVE ~4.4us/tile vs ~4.7 baseline.
            def emit_spans(t, dsb, base):
                for h in range(2):
                    ps = psp.tile([128, 2048], F32, name="ps_main", tag="ps_main")
                    for g in range(4):
                        col0 = 2048 * h + 512 * g
                        nc.tensor.matmul(
                            ps[:, 512 * g : 512 * (g + 1)],
                            zp_pk[32 * g : 32 * g + K, 128 * t : 128 * (t + 1)],
                            zg_pk[32 * g : 32 * g + K, col0 : col0 + 512],
                            start=True,
                            stop=True,
                            tile_position=(32 * g, 0),
                        )
                    nc.scalar.activation(
                        out=dsb[:, base + 2048 * h : base + 2048 * (h + 1)],
                        in_=ps,
                        func=mybir.ActivationFunctionType.Relu,
                    )

            def fold_rows(dsb, n, t0):
                # deep bf16 fold-tree over n packed tiles (3D APs keep 2x)
                cur = dsb.rearrange("p (i m) -> p i m", m=M)[:, 0:n, :]
                w = M // 2
                for ft in ftiles:
                    nc.vector.tensor_tensor(
                        out=ft[:, 0:n, :],
                        in0=cur[:, :, 0:w],
                        in1=cur[:, :, w : 2 * w],
                        op=mybir.AluOpType.min,
                    )
                    cur = ft[:, 0:n, :]
                    w //= 2
                nc.vector.tensor_reduce(
                    out=rowmins[:, t0 : t0 + n],
                    in_=cur,
                    axis=mybir.AxisListType.X,
                    op=mybir.AluOpType.min,
                )

            def col_update(dsb, i0):
                nc.vector.tensor_tensor(
                    out=colacc2_v,
                    in0=dsb.rearrange("p (i m) -> p i m", m=M)[:, i0 : i0 + 2, :],
                    in1=colacc2_v,
                    op=mybir.AluOpType.min,
                )

            if True:
                # t = 0,1 write the two col accumulators directly
                emit_spans(0, colacc2, 0)
                emit_spans(1, colacc2, M)
                fold_rows(colacc2, 2, 0)
                # remaining 30 tiles as 7 quads + final pair
                for t0 in range(2, NT, 4):
                    n = min(4, NT - t0)
                    dsb = dsbp.tile([128, 4 * M], BF16, name="dsb", tag="dsb")
                    for i in range(n):
                        emit_spans(t0 + i, dsb, i * M)
                    col_update(dsb, 0)
                    fold_rows(dsb, n, t0)
                    if n == 4:
                        col_update(dsb, 2)
                # merge the paired accumulators for the transpose tail
                nc.vector.tensor_tensor(
                    out=colacc,
                    in0=colacc2[:, 0:M],
                    in1=colacc2[:, M : 2 * M],
                    op=mybir.AluOpType.min,
                )

            # ---- tail: min over partitions of colacc via PE transpose ----
            colmins = persist.tile([128, NT], F32)
            if True:
                psp2 = psp
                for g in range(4):  # 4 groups of 8 [128,128] blocks
                    pst = psp2.tile([128, 1024], BF16, name="ps_tr", tag="ps_main")
                    for k in range(8):
                        b = 8 * g + k
                        nc.tensor.matmul(
                            pst[:, 128 * k : 128 * (k + 1)],
                            colacc[:, 128 * b : 128 * (b + 1)],
                            ident16,
                            is_transpose=True,
                            start=True,
                            stop=True,
                        )
                    nc.vector.tensor_reduce(
                        out=colmins[:, 8 * g : 8 * (g + 1)],
                        in_=pst.rearrange("p (k i) -> p k i", i=128),
                        axis=mybir.AxisListType.X,
                        op=mybir.AluOpType.min,
                    )

                # ---- final sums (minima already clamped: ACT relu-copies) ----
                rowsum = persist.tile([128, 1], F32)
                colsum = persist.tile([128, 1], F32)
                nc.vector.tensor_reduce(
                    out=rowsum,
                    in_=rowmins,
                    axis=mybir.AxisListType.X,
                    op=mybir.AluOpType.add,
                )
                nc.vector.tensor_reduce(
                    out=colsum,
                    in_=colmins,
                    axis=mybir.AxisListType.X,
                    op=mybir.AluOpType.add,
                )
                total = persist.tile([128, 1], F32)
                nc.vector.tensor_add(total, rowsum, colsum)
                ps_s = psp2.tile([1, 1], F32, name="ps_s", tag="ps_main")
                ones = nc.const_aps.tensor(1.0, (128, 1))
                nc.tensor.matmul(ps_s, ones, total, start=True, stop=True)
                res_sb = persist.tile([1, 1], F32)
                nc.scalar.mul(res_sb, ps_s, 1.0 / float(N))
                nc.sync.dma_start(out=out_d.ap(), in_=res_sb)
            foldp.release()
            psp.release()

    _split_multi_waits(nc)
    return nc


_NC = None


def _get_nc():
    global _NC
    if _NC is None:
        _NC = build_nc()
    return _NC


def _ensure_ntff_hook():
    """Register the axon NTFF profiling hook if the container's antenv stub
    lacks axon_hooks (trace support; harmless to skip)."""
    import types

    try:
        import antenv
    except ImportError:
        return
    if not hasattr(antenv, "axon_hooks") or not hasattr(
        getattr(antenv, "axon_hooks", None), "get_axon_ntff_profile_hook"
    ):
        mod = types.ModuleType("antenv.axon_hooks")
        mod._h = None
        mod.set_axon_ntff_profile_hook = lambda h: setattr(mod, "_h", h)
        mod.get_axon_ntff_profile_hook = lambda: mod._h
        sys.modules["antenv.axon_hooks"] = mod
        antenv.axon_hooks = mod
    from antenv import axon_hooks

    if axon_hooks.get_axon_ntff_profile_hook() is None:
        try:
            from trn_agent_boot.trn_boot import _ntff_profile_via_ctypes

            hook = _ntff_profile_via_ctypes("/opt/axon/libaxon_pjrt.so")
            if hook is not None:
                axon_hooks.set_axon_ntff_profile_hook(hook)
        except Exception:
            pass


def kernel(pred_points: np.ndarray, gt_points: np.ndarray, _want_trace: bool = False):
    pred = np.ascontiguousarray(np.asarray(pred_points, dtype=np.float32))
    gt = np.ascontiguousarray(np.asarray(gt_points, dtype=np.float32))
    assert pred.shape == (B, N, D) and gt.shape == (B, M, D)

    in_maps = []
    for b in range(B):
        p, g = pred[b], gt[b]
        in_maps.append(
            {
                "predW": np.ascontiguousarray(
                    p.reshape(NT, 128, 3).transpose(1, 0, 2).reshape(128, 3 * NT)
                ),
                "gtW": np.ascontiguousarray(
                    g.reshape(NT, 128, 3).transpose(1, 0, 2).reshape(128, 3 * NT)
                ),
            }
        )

    nc = _get_nc()
    if _want_trace:
        _ensure_ntff_hook()
    res = run_bass_kernel_spmd(nc, in_maps, core_ids=list(range(B)), trace=_want_trace)
    per_core = np.array([r["out"][0, 0] for r in res.results], dtype=np.float64)
    loss = np.float32(per_core.mean())
    if _want_trace:
        return loss, res
    return loss



# revision 13
# speedup vs baseline: 1.1536x; 1.1536x over previous
"""Chamfer loss kernel for Trainium2 (8 NeuronCores, data-parallel over batch).

Problem: B=8, N=M=4096, D=3 fp32 point clouds.
  loss = mean_b mean_n min_m ||p_bn - g_bm||^2  +  mean_b mean_m min_n ||.||^2
  (squared euclidean, clamped at 0, matching pytorch3d norm=2 semantics)

Strategy (one batch element per core):
  - Distance tiles d[n, m] = ||p||^2 + ||g||^2 - 2 p.g come from K=7 float32r
    matmuls on the PE. float32r keeps ~12 mantissa bits, so the points are
    rounded once (consistently) and the squared norms are computed FROM the
    rounded points, split into hi+lo f32r rows so the norm contribution keeps
    full fp32 precision. Row pairing (lhsT row k x rhs row k):
       k0: 1 * |g|^2_hi   k1: 1 * |g|^2_lo
       k2: |p|^2_hi * 1   k3: |p|^2_lo * 1
       k4-6: (-2 p_d) * g_d
    The resulting d error is only the point-rounding perturbation (~2e-4
    relative on coordinates), which moves the final loss by ~1e-5 relative.
  - The K=7 strips are replicated at PE row-group partition bases
    {0, 32, 64, 96} and 4 matmuls run CONCURRENTLY via tile_position row
    tiling (measured ~59us for all 256 [128, 512] tiles vs 456us for fp32).
  - Row minima: fused vector.tensor_scalar reads each 4-bank PSUM span once:
    out = relu(d) cast to bf16 into SBUF (feeds the column path), accum_out =
    running min along free, chained across the two spans per n-tile.
  - Column minima: bf16 tensor_tensor min into colacc[128, 4096], then a
    PE-transpose + free-axis reduce tail for the partition-axis min.
  - Per-core scalar output (cham_x_b + cham_y_b); the host averages the 8
    per-core scalars (the data-parallel gather).

All arithmetic happens on-chip; the host only reshapes/transposes inputs
(layout) and averages the per-core partial losses (unshard).
"""

import os
import sys

import numpy as np

sys.path.insert(0, "/opt/trn_rl_repo")

import bass_rust
import concourse.bass as bass
import concourse.mybir as mybir
from concourse.bass_utils import run_bass_kernel_spmd
from concourse.masks import make_identity
from concourse.tile import TileContext

B, N, M, D = 8, 4096, 4096, 3
NT = N // 128  # 32 n-tiles
K = 7
F32 = mybir.dt.float32
F32R = mybir.dt.float32r
BF16 = mybir.dt.bfloat16
BIG = 3.0e38

# ---------------------------------------------------------------------------
# walrus in this container rejects >1 sync-wait per instruction; spill the
# extras onto engine-matched NoOps placed immediately before the instruction.
_nop_counter = [0]


def _split_multi_waits(nc):
    for func in nc.m.functions:
        for bb in func.blocks:
            out = []
            dirty = False
            for inst in bb.instructions:
                si = inst.sync_info
                if si is not None and len(si.on_wait) > 1:
                    waits = list(si.on_wait)
                    for w in waits[:-1]:
                        _nop_counter[0] += 1
                        nop = mybir.InstNoOp(
                            name=f"I-waitsplit-{_nop_counter[0]}", ins=[], outs=[]
                        )
                        nop.engine = inst.engine
                        nop.sync_info = bass_rust.SyncInfo(on_wait=[w], on_update=[])
                        out.append(nop)
                    inst.sync_info = bass_rust.SyncInfo(
                        on_wait=[waits[-1]], on_update=list(si.on_update)
                    )
                    dirty = True
                out.append(inst)
            if dirty:
                bb.instructions = out
    return nc


# ---------------------------------------------------------------------------


_PREP_WR = {}


def _build_prep_side(nc, tc, pool, zpk, w_dram, ident, scale, hi_row, lo_row, c_row, q, psp):
    """Build one side's K=7 rows in row group 0 of zpk [96+K, 4096] f32r.

    Everything derives from the wide input layout [128, 96] (point 128t+p at
    partition p, cols 3t..3t+2): rounded coords via PE-transpose, squared
    norms (of the rounded points) hi/lo-split, both flattened into n-order
    by ONE reshape DMA each on this side's HWDGE queue (requires lo_row ==
    hi_row + 1). Row groups 1-3 are filled by the replication hops in
    build_nc; keeping the DMA count tiny is what keeps the prep ramp short.
    """
    nm = w_dram.name
    wr = _PREP_WR[nm]  # rounded+scaled wide input, loaded up front

    if True:
        # --- coords: transpose wr -> [96, 128], one 3-row reshape DMA ---
        tw_ps = psp.tile([128, 128], F32, name=f"twps_{nm}", tag="ps_main")
        nc.tensor.matmul(
            tw_ps[0 : 3 * NT, :],
            wr.bitcast(F32),
            ident,
            is_transpose=True,
            start=True,
            stop=True,
        )
        tw = pool.tile([3 * NT, 128], F32R, name=f"tw_{nm}")
        nc.vector.tensor_copy(tw, tw_ps[0 : 3 * NT, :])
        tw_d = tw.rearrange("(t d) p -> d t p", d=3)
        for d in range(3):
            q.dma_start(out=zpk[c_row + d : c_row + d + 1, :], in_=tw_d[d])

        # --- norms of rounded points ---
        wsq = pool.tile([128, 3 * NT], F32, name=f"wsq_{nm}")
        nc.vector.tensor_mul(wsq, wr, wr)
        norms = pool.tile([128, NT], F32, name=f"norms_{nm}")
        nc.vector.tensor_reduce(
            out=norms,
            in_=wsq.rearrange("p (t d) -> p t d", d=3),
            axis=mybir.AxisListType.X,
            op=mybir.AluOpType.add,
        )
        if scale != 1.0:
            # norms of scale*p -> divide by scale^2 (exact for powers of 2)
            nc.vector.tensor_scalar(
                out=norms,
                in0=norms,
                scalar1=1.0 / (scale * scale),
                scalar2=None,
                op0=mybir.AluOpType.mult,
            )
        nh = pool.tile([128, NT], F32R, name=f"nh_{nm}")
        nc.vector.tensor_copy(nh, norms)
        nl_f = pool.tile([128, NT], F32, name=f"nlf_{nm}")
        nc.vector.tensor_sub(nl_f, norms, nh.bitcast(F32))
        nl = pool.tile([128, NT], F32R, name=f"nl_{nm}")
        nc.vector.tensor_copy(nl, nl_f)
        assert lo_row == hi_row + 1
        tn2 = pool.tile([2 * NT, 128], F32R, name=f"tn2_{nm}")
        for i, src in enumerate((nh, nl)):
            tn_ps = psp.tile([128, 128], F32, name=f"tnps_{nm}_{i}", tag="ps_main")
            nc.tensor.matmul(
                tn_ps[0:NT, :],
                src.bitcast(F32),
                ident,
                is_transpose=True,
                start=True,
                stop=True,
            )
            nc.vector.tensor_copy(tn2[NT * i : NT * (i + 1), :], tn_ps[0:NT, :])
        q.dma_start(out=zpk[hi_row : hi_row + 2, :], in_=tn2)


def build_nc():
    nc = bass.Bass("TRN2")
    predW = nc.dram_tensor("predW", [128, 3 * NT], F32, kind="ExternalInput")
    gtW = nc.dram_tensor("gtW", [128, 3 * NT], F32, kind="ExternalInput")
    out_d = nc.dram_tensor("out", [1, 1], F32, kind="ExternalOutput")

    with TileContext(nc) as tc:
        with (
            tc.tile_pool(name="persist", bufs=1) as persist,
            tc.tile_pool(name="dsb", bufs=2) as dsbp,
        ):
            # load + round both wide inputs first — everything derives from
            # them, so they must not queue behind prep DMAs
            for w_dram, scale, q in ((predW, -2.0, nc.sync), (gtW, 1.0, nc.scalar)):
                w_in = persist.tile([128, 3 * NT], F32, name=f"w_{w_dram.name}")
                q.dma_start(out=w_in, in_=w_dram.ap())
                wr_t = persist.tile([128, 3 * NT], F32R, name=f"wr_{w_dram.name}")
                nc.scalar.mul(out=wr_t, in_=w_in, mul=scale)
                _PREP_WR[w_dram.name] = wr_t
            # identity zero-fill on DVE so GPSIMD's single queue only does
            # the diagonal writes (keeps the prep critical path short)
            ident = persist.tile([128, 128], F32)
            nc.vector.memset(ident, 0.0)
            make_identity(nc, ident, nomemset=True)
            ident16 = persist.tile([128, 128], BF16)
            nc.vector.memset(ident16, 0.0)
            make_identity(nc, ident16, nomemset=True)

            # K=7 operand strips, replicated at the 4 PE row-group bases so
            # 4 matmuls stream concurrently (PE ~59us vs ~200us unpacked
            # in-situ; LDWEIGHTS overlaps across row groups).
            zp_pk = persist.tile([96 + K, N], F32R)
            zg_pk = persist.tile([96 + K, M], F32R)
            # ones rows (Memset can't target f32r; compute ops can't start at
            # unaligned partitions): rounded ones strip -> broadcast DMA.
            # ones rows, built wide (128 lanes, ~100ns) then DMA'd out
            ones_w = persist.tile([128, 64], F32)
            nc.vector.memset(ones_w, 1.0)
            ones_wr = persist.tile([128, 64], F32R)
            nc.scalar.copy(ones_wr, ones_w)
            nc.sync.dma_start(out=zp_pk[0:2, :], in_=ones_wr)
            nc.scalar.dma_start(out=zg_pk[2:4, :], in_=ones_wr)
            # zp rows: [1, 1, nPh, nPl, -2p0, -2p1, -2p2]
            # zg rows: [nGh, nGl, 1, 1, g0, g1, g2]
            psp = tc.alloc_tile_pool(name="psum_all", bufs=2, space="PSUM")
            _build_prep_side(
                nc, tc, persist, zp_pk, predW, ident, -2.0, 2, 3, 4,
                nc.sync, psp,
            )
            _build_prep_side(
                nc, tc, persist, zg_pk, gtW, ident, 1.0, 0, 1, 4,
                nc.scalar, psp,
            )


            rowmins = persist.tile([128, NT], F32)
            # two independent column accumulators (updated as one [128,2,M]
            # tensor_tensor, halving the op count) merged once at the end
            colacc2 = persist.tile([128, 2 * M], BF16)
            colacc = persist.tile([128, M], BF16)
            colacc2_v = colacc2.rearrange("p (i m) -> p i m", m=M)
            # fold-tree intermediates: written+read only by the DVE (in-order)
            # so single buffers suffice; sized for the max group of 4 tiles
            foldp = tc.alloc_tile_pool(name="foldp", bufs=1)
            ftiles = [
                foldp.tile([128, 4, w], BF16, name=f"f{w}") for w in (2048, 1024, 512, 256, 128, 64)
            ]

            # ---- main loop: 32 n-tiles x 2 half-spans of [128, 2048];
            #      each span = 4 row-group-packed concurrent matmuls ----
            # Pipeline per n-tile: PE (4 packed MMs per half-span) -> ACT
            # relu-copies PSUM->SBUF bf16 (clamp fused, so no tail clamps)
            # -> DVE row path: deep bf16 fold-tree (2x mode) over groups of
            # 4 tiles down to width 64 before the (1x-rate) tensor_reduce;
            # col path: [128,2,M] tensor_tensor min into the paired
            # accumulators. Steady-state DVE ~4.4us/tile vs ~4.7 baseline.
            def emit_spans(t, dsb, base):
                for h in range(2):
                    ps = psp.tile([128, 2048], F32, name="ps_main", tag="ps_main")
                    for g in range(4):
                        col0 = 2048 * h + 512 * g
                        nc.tensor.matmul(
                            ps[:, 512 * g : 512 * (g + 1)],
                            zp_pk[0:K, 128 * t : 128 * (t + 1)],
                            zg_pk[0:K, col0 : col0 + 512],
                            start=True,
                            stop=True,
                        )
                    nc.scalar.activation(
                        out=dsb[:, base + 2048 * h : base + 2048 * (h + 1)],
                        in_=ps,
                        func=mybir.ActivationFunctionType.Relu,
                    )

            def fold_rows(dsb, n, t0):
                # deep bf16 fold-tree over n packed tiles (3D APs keep 2x)
                cur = dsb.rearrange("p (i m) -> p i m", m=M)[:, 0:n, :]
                w = M // 2
                for ft in ftiles:
                    nc.vector.tensor_tensor(
                        out=ft[:, 0:n, :],
                        in0=cur[:, :, 0:w],
                        in1=cur[:, :, w : 2 * w],
                        op=mybir.AluOpType.min,
                    )
                    cur = ft[:, 0:n, :]
                    w //= 2
                nc.vector.tensor_reduce(
                    out=rowmins[:, t0 : t0 + n],
                    in_=cur,
                    axis=mybir.AxisListType.X,
                    op=mybir.AluOpType.min,
                )

            def col_update(dsb, i0):
                nc.vector.tensor_tensor(
                    out=colacc2_v,
                    in0=dsb.rearrange("p (i m) -> p i m", m=M)[:, i0 : i0 + 2, :],
                    in1=colacc2_v,
                    op=mybir.AluOpType.min,
                )

            if True:
                # t = 0,1 write the two col accumulators directly
                emit_spans(0, colacc2, 0)
                emit_spans(1, colacc2, M)
                fold_rows(colacc2, 2, 0)
                # remaining 30 tiles as 7 quads + final pair
                for t0 in range(2, NT, 4):
                    n = min(4, NT - t0)
                    dsb = dsbp.tile([128, 4 * M], BF16, name="dsb", tag="dsb")
                    for i in range(n):
                        emit_spans(t0 + i, dsb, i * M)
                    col_update(dsb, 0)
                    fold_rows(dsb, n, t0)
                    if n == 4:
                        col_update(dsb, 2)
                # merge the paired accumulators for the transpose tail
                nc.vector.tensor_tensor(
                    out=colacc,
                    in0=colacc2[:, 0:M],
                    in1=colacc2[:, M : 2 * M],
                    op=mybir.AluOpType.min,
                )

            # ---- tail: min over partitions of colacc via PE transpose ----
            colmins = persist.tile([128, NT], F32)
            if True:
                psp2 = psp
                for g in range(4):  # 4 groups of 8 [128,128] blocks
                    pst = psp2.tile([128, 1024], BF16, name="ps_tr", tag="ps_main")
                    for k in range(8):
                        b = 8 * g + k
                        nc.tensor.matmul(
                            pst[:, 128 * k : 128 * (k + 1)],
                            colacc[:, 128 * b : 128 * (b + 1)],
                            ident16,
                            is_transpose=True,
                            start=True,
                            stop=True,
                        )
                    nc.vector.tensor_reduce(
                        out=colmins[:, 8 * g : 8 * (g + 1)],
                        in_=pst.rearrange("p (k i) -> p k i", i=128),
                        axis=mybir.AxisListType.X,
                        op=mybir.AluOpType.min,
                    )

                # ---- final sums (minima already clamped: ACT relu-copies) ----
                rowsum = persist.tile([128, 1], F32)
                colsum = persist.tile([128, 1], F32)
                nc.vector.tensor_reduce(
                    out=rowsum,
                    in_=rowmins,
                    axis=mybir.AxisListType.X,
                    op=mybir.AluOpType.add,
                )
                nc.vector.tensor_reduce(
                    out=colsum,
                    in_=colmins,
                    axis=mybir.AxisListType.X,
                    op=mybir.AluOpType.add,
                )
                total = persist.tile([128, 1], F32)
                nc.vector.tensor_add(total, rowsum, colsum)
                ps_s = psp2.tile([1, 1], F32, name="ps_s", tag="ps_main")
                ones = nc.const_aps.tensor(1.0, (128, 1))
                nc.tensor.matmul(ps_s, ones, total, start=True, stop=True)
                res_sb = persist.tile([1, 1], F32)
                nc.scalar.mul(res_sb, ps_s, 1.0 / float(N))
                nc.sync.dma_start(out=out_d.ap(), in_=res_sb)
            foldp.release()
            psp.release()

    _split_multi_waits(nc)
    return nc


_NC = None


def _get_nc():
    global _NC
    if _NC is None:
        _NC = build_nc()
    return _NC


def _ensure_ntff_hook():
    """Register the axon NTFF profiling hook if the container's antenv stub
    lacks axon_hooks (trace support; harmless to skip)."""
    import types

    try:
        import antenv
    except ImportError:
        return
    if not hasattr(antenv, "axon_hooks") or not hasattr(
        getattr(antenv, "axon_hooks", None), "get_axon_ntff_profile_hook"
    ):
        mod = types.ModuleType("antenv.axon_hooks")
        mod._h = None
        mod.set_axon_ntff_profile_hook = lambda h: setattr(mod, "_h", h)
        mod.get_axon_ntff_profile_hook = lambda: mod._h
        sys.modules["antenv.axon_hooks"] = mod
        antenv.axon_hooks = mod
    from antenv import axon_hooks

    if axon_hooks.get_axon_ntff_profile_hook() is None:
        try:
            from trn_agent_boot.trn_boot import _ntff_profile_via_ctypes

            hook = _ntff_profile_via_ctypes("/opt/axon/libaxon_pjrt.so")
            if hook is not None:
                axon_hooks.set_axon_ntff_profile_hook(hook)
        except Exception:
            pass


def kernel(pred_points: np.ndarray, gt_points: np.ndarray, _want_trace: bool = False):
    pred = np.ascontiguousarray(np.asarray(pred_points, dtype=np.float32))
    gt = np.ascontiguousarray(np.asarray(gt_points, dtype=np.float32))
    assert pred.shape == (B, N, D) and gt.shape == (B, M, D)

    in_maps = []
    for b in range(B):
        p, g = pred[b], gt[b]
        in_maps.append(
            {
                "predW": np.ascontiguousarray(
                    p.reshape(NT, 128, 3).transpose(1, 0, 2).reshape(128, 3 * NT)
                ),
                "gtW": np.ascontiguousarray(
                    g.reshape(NT, 128, 3).transpose(1, 0, 2).reshape(128, 3 * NT)
                ),
            }
        )

    nc = _get_nc()
    if _want_trace:
        _ensure_ntff_hook()
    res = run_bass_kernel_spmd(nc, in_maps, core_ids=list(range(B)), trace=_want_trace)
    per_core = np.array([r["out"][0, 0] for r in res.results], dtype=np.float64)
    loss = np.float32(per_core.mean())
    if _want_trace:
        return loss, res
    return loss

